# revision 39
# baseline (speedup 1.0000x reference)
"""TRN2 Bass kernel: masked-centroid squared distances (8 NeuronCores, SPMD).

Reference computation (fp32):
    C = U^T X / B                          [K, D]   (B=512, K=512, D=1024)
    mask = round(clip(M, 0, 1)) = (M > 0.5)
    D_out[b, k] = sum_d mask[k,d] * (X[b,d] - C[k,d])^2

Algebraic expansion (mask^2 = mask):
    D_out[b,k] = sum_d mask*X^2  - 2*sum_d (mask*C)*X  + sum_d mask*C^2

Sharding: each of the 8 cores owns a 64-row shard of C / mask / D_out^T
(out_dim shard) -> every core needs full X but no collectives at all.

Per-core dataflow (d-major layout, d on partitions for the big matmuls):
    Ĉᵀraw[d,k] = sum_b X[b,d] U_s[b,k]   (PE fp8, computed directly in the
        transposed layout: lhsT = X b-chunk, rhs = U_s b-chunk -> [128, 64]
        chunks, j-major accumulation groups split over two PSUM banks)
    maskᵀ = (Mᵀtrunc >= 0.5)  (Mᵀ arrives host-pre-packed, truncated to
        bf16 — exactly equivalent to fp32 (M > 0.5); DVE is_ge -> bf16)
    CMᵀ = (Ĉᵀraw * -1/256) * maskᵀ = -2*mask*C    (fused DVE stt -> bf16)
    Gᵀ  = (Ĉᵀraw * -1/256) * CMᵀ  = 4*mask*C^2   (fused DVE stt -> bf16)
    X2ᵀ = XTᵀ * XTᵀ      (per-d-chunk squares split across DVE/ACT -> bf16)
    Dᵀ  += maskᵀ.T @ X2ᵀ   (T1)     (PE bf16, one accum group [64, 512])
    Dᵀ  += CMᵀ.T  @ XTᵀ    (-2*T2)
    t3raw[64,1] = G.T @ 1  (near-free N=1 PE matmuls into a PSUM vector)
    Dᵀ_out = Dᵀ + 0.25*t3raw   (t3 folded into the PSUM->SBUF copy as a
        per-partition scalar add on DVE) -> DMA out [64, 512] f32

Scheduling (tuned against the TimelineSim cost model / HAM clock-gate):
  - single HWDGE DMA stream, arrival order ms, xba(+U_s baked in), xt01,
    xbb, xt23, xt45, xt6, xt7 — the centroid/mask ladders unlock first and
    the T1/T2 moving operand streams in last with small final chunks;
  - warm-up dummy matmuls keep the PE p-state ramping while DMAs land;
  - T2 matmuls lead the accumulation group (their rhs needs no square);
    T1-j7 closes the group since its square is the last dependency.

Precision: X enters the distance terms in bf16 (both layouts).  X and U
enter the centroid matmul in fp8e4m3 — C is ~40x smaller than X and only
enters D through second-order terms, so fp8's ~4% element error adds ~1e-4
relative error while cutting those operands' DMA 4x.  M ships as
round-toward-zero bf16: (trunc(M) >= 0.5) == (M > 0.5) for every fp32 value
except M == 0.5 exactly, which the host nudges one ulp down — the mask is
bit-identical to the reference.

Host does layout/dtype prep only (casts, transposes, sharding, gather);
all FLOPs of the algorithm run on device.

Measured: relative error 1.25e-3 vs fp32 reference on all 8 cores;
TimelineSim cost model 13.75 us/core (first correct version was 21.4 us).
"""

import numpy as np

BATCH = 512
OUT_DIM = 512
IN_DIM = 1024
N_CORES = 8
KS = OUT_DIM // N_CORES  # 64 centroid rows per core

_CACHE = {}


def build_module(num_devices: int = N_CORES):
    """Build + compile the Bass module (same SPMD program for every core)."""
    import concourse.bacc as bacc
    import concourse.mybir as mybir
    from concourse import tile

    if num_devices in _CACHE:
        return _CACHE[num_devices]

    fp32 = mybir.dt.float32
    bf16 = mybir.dt.bfloat16
    fp8 = mybir.dt.float8e4
    Alu = mybir.AluOpType
    Act = mybir.ActivationFunctionType

    nc = bacc.Bacc("TRN2", target_bir_lowering=False, debug=False,
                   num_devices=num_devices)

    NB = BATCH // 128   # 4 b-chunks
    ND = IN_DIM // 128  # 8 d-chunks

    # xb arrives d-chunk-major, host-packed into two flat fp8 blocks:
    # xba[p, 1024*m + 256*i + dd] = X[128*i + p, 256*m + dd] for m in {0,1},
    # plus U_s baked into its last 256 cols (xba[p, 2048 + 64*i + k] =
    # U[128*i + p, 64*core + k]); xbb covers m in {2,3}.
    xba = nc.dram_tensor("xba", [128, 2 * IN_DIM + 256], fp8,
                         kind="ExternalInput").ap()
    xbb = nc.dram_tensor("xbb", [128, 2 * IN_DIM], fp8,
                         kind="ExternalInput").ap()
    xt = nc.dram_tensor("xt", [IN_DIM, BATCH], bf16, kind="ExternalInput").ap()
    # mask source arrives pre-transposed+packed and TRUNCATED to bf16:
    # ms[p, 64*j + k] = trunc_bf16(M_s[k, 128*j + p]).  Truncation toward
    # zero makes (ms >= 0.5) == (M > 0.5) exactly, except M == 0.5 which the
    # host nudges down one ulp.  Halves the mask DMA.
    ms = nc.dram_tensor("ms", [128, 512], bf16, kind="ExternalInput").ap()
    dt_out = nc.dram_tensor("dt", [KS, BATCH], fp32, kind="ExternalOutput").ap()

    with tile.TileContext(nc) as tc:
        with (
            tc.tile_pool(name="const", bufs=1) as constp,
            tc.tile_pool(name="xbp", bufs=1) as xbp,
            tc.tile_pool(name="xtp", bufs=1) as xtp,
            tc.tile_pool(name="x2tp", bufs=1) as x2tp,
            tc.tile_pool(name="smal", bufs=1) as smal,
            tc.tile_pool(name="psum", bufs=1, space="PSUM") as psp,
        ):
            # ---- constants (all on DVE; Pool/GPSIMD stays fully idle)
            wtile = constp.tile([128, 512], bf16, tag="wtile")
            nc.vector.memset(wtile[:, :], 0.0)
            ones_col = constp.tile([128, 1], bf16, tag="ones")
            nc.vector.memset(ones_col[:, :], 1.0)

            # ---- DMA in.  One HWDGE stream, engine-bound; order tuned so
            # each consumer ladder unlocks earliest: mask source first (its
            # is_gt gates cmt), then centroid operands, xt last.
            ms_sb = smal.tile([128, 512], bf16, tag="ms")
            nc.sync.dma_start(ms_sb[:, :], ms[:, :])

            xba_sb = xbp.tile([128, 2 * IN_DIM + 256], fp8, tag="xba")
            nc.sync.dma_start(xba_sb[:, :], xba[:, :])
            xb_t = [xba_sb, None]
            us_sb = xba_sb  # U_s lives at cols [2048, 2304)

            xt_q = [xtp.tile([128, 2 * BATCH], bf16, tag=f"xtq{q}", name=f"xtq{q}")
                    for q in range(3)]
            xt_s = [xtp.tile([128, BATCH], bf16, tag=f"xts{j}", name=f"xts{j}")
                    for j in (6, 7)]

            def dma_xtq(q):
                nc.sync.dma_start(
                    xt_q[q][:, :].rearrange("p (r b) -> p r b", r=2),
                    xt[256 * q:256 * (q + 1), :].rearrange("(r p) b -> p r b", p=128),
                )

            dma_xtq(0)
            xbb_sb = xbp.tile([128, 2 * IN_DIM], fp8, tag="xbb")
            nc.sync.dma_start(xbb_sb[:, :], xbb[:, :])
            xb_t[1] = xbb_sb
            dma_xtq(1)
            dma_xtq(2)
            for idx, j in enumerate((6, 7)):
                nc.sync.dma_start(xt_s[idx][:, :], xt[128 * j:128 * (j + 1), :])

            def xt_slice(j):
                if j < 6:
                    return xt_q[j // 2][:, 512 * (j % 2):512 * (j % 2 + 1)]
                return xt_s[j - 6][:, :]

            # ---- PE warm-up: dummy matmuls (no data deps) ramp the PE clock
            # while DMAs land; they write psum_d which T1-j0 later resets.
            psum_d = psp.tile([64, 512], fp32, tag="pd")
            psum_w = psp.tile([64, 512], fp32, tag="pw")

            def dummy_mm(n=512):
                nc.tensor.matmul(psum_w[:, 0:n], wtile[:, 0:64], wtile[:, 0:n],
                                 start=True, stop=True)

            for _ in range(5):
                dummy_mm()

            # ---- maskᵀ = (Mᵀ > 0.5)
            maskt = smal.tile([128, 512], bf16, tag="maskt")
            nc.vector.tensor_scalar(maskt[:, :], ms_sb[:, :], 0.5, None,
                                    Alu.is_ge)

            # ---- Ĉᵀraw[d,k] direct: per d-chunk j accumulate over b-chunks.
            # lhsT = X[b-chunk, d-chunk] (fp8), rhs = U_s[b-chunk] (fp8).
            # j-major (one pending PSUM accumulation group at a time); each
            # xb half covers 4 whole j-groups, so pacing is preserved.
            psum_ct = [psp.tile([128, 256], fp32, tag=f"pct{x}", name=f"pct{x}")
                       for x in range(2)]
            for j in range(ND):
                a, mm = divmod(j, 4)  # xb half a; j-major within each bank
                base = 1024 * (mm // 2) + 128 * (mm % 2)
                for i in range(NB):
                    nc.tensor.matmul(
                        psum_ct[a][:, 64 * mm:64 * (mm + 1)],
                        xb_t[a][:, base + 256 * i:base + 256 * i + 128],
                        us_sb[:, 2048 + KS * i:2048 + KS * (i + 1)],
                        start=(i == 0), stop=(i == NB - 1),
                    )
            dummy_mm(128)

            # ---- X2ᵀ squares as per-j [128, 512] units (each feeds exactly
            # one T1 matmul) alternating DVE/ACT, plus fused CM/G products.
            x2t_q = [x2tp.tile([128, 2 * BATCH], bf16, tag=f"x2q{q}", name=f"x2q{q}")
                     for q in range(3)]
            x2t_s = [x2tp.tile([128, BATCH], bf16, tag=f"x2s{j}", name=f"x2s{j}")
                     for j in (6, 7)]

            def x2t_slice(j):
                if j < 6:
                    return x2t_q[j // 2][:, 512 * (j % 2):512 * (j % 2 + 1)]
                return x2t_s[j - 6][:, :]

            SQ_ON_ACT = {1, 3, 5}
            for j in range(ND):
                dst, srcap = x2t_slice(j), xt_slice(j)
                if j in SQ_ON_ACT:
                    nc.scalar.activation(dst, srcap, Act.Square)
                else:
                    nc.vector.tensor_tensor(dst, srcap, srcap, Alu.mult)

            cmt = smal.tile([128, 512], bf16, tag="cmt")
            g_sb = smal.tile([128, 512], bf16, tag="g")
            for hh in range(2):
                sl = slice(256 * hh, 256 * (hh + 1))
                nc.vector.scalar_tensor_tensor(cmt[:, sl], psum_ct[hh][:, :],
                                               -1.0 / 256.0, maskt[:, sl],
                                               Alu.mult, Alu.mult)
            for hh in range(2):
                sl = slice(256 * hh, 256 * (hh + 1))
                nc.vector.scalar_tensor_tensor(g_sb[:, sl], psum_ct[hh][:, :],
                                               -1.0 / 256.0, cmt[:, sl],
                                               Alu.mult, Alu.mult)

            # ---- Dᵀ accumulation: one PSUM group; T2 (rhs = xt directly)
            # leads since cmt unlocks before the squares; T1-j follows its
            # square.  t3 = colsum(G)/4 accumulates separately as a [64, 1]
            # PSUM vector via near-free N=1 matmuls and is folded into the
            # final PSUM->SBUF copy as a per-partition scalar add.
            def t1(j, start=False, stop=False):
                nc.tensor.matmul(psum_d[:, :], maskt[:, 64 * j:64 * (j + 1)],
                                 x2t_slice(j), start=start, stop=stop)

            def t2(j, start=False, stop=False):
                nc.tensor.matmul(psum_d[:, :], cmt[:, 64 * j:64 * (j + 1)],
                                 xt_slice(j), start=start, stop=stop)

            psum_t3 = psp.tile([64, 1], fp32, tag="pt3")
            d_sb = smal.tile([64, 512], fp32, tag="d")
            t3s = smal.tile([64, 1], fp32, tag="t3s")

            t2(0, start=True)
            t2(1)
            t2(2)
            t2(3)
            t1(0)
            t2(4)
            t2(5)
            t1(1)
            t1(2)
            for j in range(ND):
                nc.tensor.matmul(psum_t3[:, :], g_sb[:, 64 * j:64 * (j + 1)],
                                 ones_col[:, :],
                                 start=(j == 0), stop=(j == ND - 1))
            t1(3)
            t2(6)
            t2(7)
            t1(4)
            t1(5)
            t1(6)
            t1(7, stop=True)

            nc.scalar.activation(t3s[:, :], psum_t3[:, :], Act.Copy, scale=0.25)
            nc.vector.tensor_scalar(d_sb[:, :], psum_d[:, :], t3s[:, 0:1], None,
                                    Alu.add)
            nc.sync.dma_start(dt_out[:, :], d_sb[:, :])

    nc.compile()
    _CACHE[num_devices] = nc
    return nc


def kernel(X: np.ndarray, U: np.ndarray, M: np.ndarray) -> np.ndarray:
    import ml_dtypes
    from concourse import bass_utils

    X = np.asarray(X, dtype=np.float32)
    U = np.asarray(U, dtype=np.float32)
    M = np.asarray(M, dtype=np.float32)
    assert X.shape == (BATCH, IN_DIM) and U.shape == (BATCH, OUT_DIM) \
        and M.shape == (OUT_DIM, IN_DIM)

    nc = build_module(N_CORES)

    bf16 = ml_dtypes.bfloat16
    fp8 = ml_dtypes.float8_e4m3
    # d-chunk-major fp8 layout: [p, 1024*m + 256*i + dd] = X[128*i + p, 256*m + dd]
    xbj = X.reshape(4, 128, 4, 256).transpose(1, 2, 0, 3).reshape(128, 4096)
    xbb_np = np.ascontiguousarray(xbj[:, 2048:4096]).astype(fp8)
    xt_np = np.ascontiguousarray(X.T).astype(bf16)
    def trunc_bf16(a):
        # round-toward-zero to bf16 so (v >= 0.5) == (a > 0.5); exact-0.5
        # inputs (mask must be 0 there) get nudged one bf16 ulp down.
        bits = np.ascontiguousarray(a, dtype=np.float32).view(np.uint32)
        v = (bits >> 16).astype(np.uint16).view(bf16).copy()
        v[a == 0.5] = np.float32(0.498046875)
        return v

    mst = [trunc_bf16(
        M[KS * c:KS * (c + 1), :].T.reshape(8, 128, KS)
        .transpose(1, 0, 2).reshape(128, 512))
        for c in range(N_CORES)]

    in_maps = []
    for c in range(N_CORES):
        usc = U[:, KS * c:KS * (c + 1)].reshape(4, 128, KS).transpose(1, 0, 2)
        xba_np = np.concatenate(
            [xbj[:, 0:2048], usc.reshape(128, 4 * KS)], axis=1).astype(fp8)
        in_maps.append({
            "xba": np.ascontiguousarray(xba_np),
            "xbb": xbb_np,
            "xt": xt_np,
            "ms": mst[c],
        })

    res = bass_utils.run_bass_kernel_spmd(nc, in_maps, core_ids=list(range(N_CORES)))

    out = np.empty((BATCH, OUT_DIM), dtype=np.float32)
    for c in range(N_CORES):
        out[:, KS * c:KS * (c + 1)] = res.results[c]["dt"].T
    return out


# revision 40
# speedup vs baseline: 1.0134x; 1.0134x over previous
"""TRN2 Bass kernel: masked-centroid squared distances (8 NeuronCores, SPMD).

Reference computation (fp32):
    C = U^T X / B                          [K, D]   (B=512, K=512, D=1024)
    mask = round(clip(M, 0, 1)) = (M > 0.5)
    D_out[b, k] = sum_d mask[k,d] * (X[b,d] - C[k,d])^2

Algebraic expansion (mask^2 = mask):
    D_out[b,k] = sum_d mask*X^2  - 2*sum_d (mask*C)*X  + sum_d mask*C^2

Sharding: each of the 8 cores owns a 64-row shard of C / mask / D_out^T
(out_dim shard) -> every core needs full X but no collectives at all.

Per-core dataflow (d-major layout, d on partitions for the big matmuls):
    Ĉᵀraw[d,k] = sum_b X[b,d] U_s[b,k]   (PE fp8, computed directly in the
        transposed layout: lhsT = X b-chunk, rhs = U_s b-chunk -> [128, 64]
        chunks, j-major accumulation groups split over two PSUM banks)
    maskᵀ = (Mᵀtrunc >= 0.5)  (Mᵀ arrives host-pre-packed, truncated to
        bf16 — exactly equivalent to fp32 (M > 0.5); DVE is_ge -> bf16)
    CMᵀ = (Ĉᵀraw * -1/256) * maskᵀ = -2*mask*C    (fused DVE stt -> bf16)
    Gᵀ  = (Ĉᵀraw * -1/256) * CMᵀ  = 4*mask*C^2   (fused DVE stt -> bf16)
    X2ᵀ = XTᵀ * XTᵀ      (per-d-chunk squares split across DVE/ACT -> bf16)
    Dᵀ  += maskᵀ.T @ X2ᵀ   (T1)     (PE bf16, one accum group [64, 512])
    Dᵀ  += CMᵀ.T  @ XTᵀ    (-2*T2)
    t3raw[64,1] = G.T @ 1  (near-free N=1 PE matmuls into a PSUM vector)
    Dᵀ_out = Dᵀ + 0.25*t3raw   (t3 folded into the PSUM->SBUF copy as a
        per-partition scalar add on DVE) -> DMA out [64, 512] f32

Scheduling (tuned against the TimelineSim cost model / HAM clock-gate):
  - single HWDGE DMA stream, arrival order ms, xba(+U_s baked in), xt01,
    xbb, xt23, xt45, xt6, xt7 — the centroid/mask ladders unlock first and
    the T1/T2 moving operand streams in last with small final chunks;
  - warm-up dummy matmuls keep the PE p-state ramping while DMAs land;
  - T2 matmuls lead the accumulation group (their rhs needs no square);
    T1-j7 closes the group since its square is the last dependency.

Precision: X enters the distance terms in bf16 (both layouts).  X and U
enter the centroid matmul in fp8e4m3 — C is ~40x smaller than X and only
enters D through second-order terms, so fp8's ~4% element error adds ~1e-4
relative error while cutting those operands' DMA 4x.  M ships as
round-toward-zero bf16: (trunc(M) >= 0.5) == (M > 0.5) for every fp32 value
except M == 0.5 exactly, which the host nudges one ulp down — the mask is
bit-identical to the reference.

Host does layout/dtype prep only (casts, transposes, sharding, gather);
all FLOPs of the algorithm run on device.

Measured: relative error 1.25e-3 vs fp32 reference on all 8 cores;
TimelineSim cost model 13.75 us/core (first correct version was 21.4 us).
"""

import numpy as np

BATCH = 512
OUT_DIM = 512
IN_DIM = 1024
N_CORES = 8
KS = OUT_DIM // N_CORES  # 64 centroid rows per core

_CACHE = {}


def build_module(num_devices: int = N_CORES):
    """Build + compile the Bass module (same SPMD program for every core)."""
    import concourse.bacc as bacc
    import concourse.mybir as mybir
    from concourse import tile

    if num_devices in _CACHE:
        return _CACHE[num_devices]

    fp32 = mybir.dt.float32
    bf16 = mybir.dt.bfloat16
    fp8 = mybir.dt.float8e4
    Alu = mybir.AluOpType
    Act = mybir.ActivationFunctionType

    nc = bacc.Bacc("TRN2", target_bir_lowering=False, debug=False,
                   num_devices=num_devices)

    NB = BATCH // 128   # 4 b-chunks
    ND = IN_DIM // 128  # 8 d-chunks

    # xb arrives d-chunk-major, host-packed into two flat fp8 blocks:
    # xba[p, 1024*m + 256*i + dd] = X[128*i + p, 256*m + dd] for m in {0,1},
    # plus U_s baked into its last 256 cols (xba[p, 2048 + 64*i + k] =
    # U[128*i + p, 64*core + k]); xbb covers m in {2,3}.
    xba = nc.dram_tensor("xba", [128, 2 * IN_DIM + 256], fp8,
                         kind="ExternalInput").ap()
    xbb = nc.dram_tensor("xbb", [128, 2 * IN_DIM], fp8,
                         kind="ExternalInput").ap()
    xt = nc.dram_tensor("xt", [IN_DIM, BATCH], bf16, kind="ExternalInput").ap()
    # mask source arrives pre-transposed+packed and TRUNCATED to bf16:
    # ms[p, 64*j + k] = trunc_bf16(M_s[k, 128*j + p]).  Truncation toward
    # zero makes (ms >= 0.5) == (M > 0.5) exactly, except M == 0.5 which the
    # host nudges down one ulp.  Halves the mask DMA.
    ms = nc.dram_tensor("ms", [128, 512], bf16, kind="ExternalInput").ap()
    # output crosses DRAM as fp16 (exactly upcast on host): D < 512 so no
    # overflow, and fp16's 2^-11 rounding adds ~5e-4 relative error.
    fp16 = mybir.dt.float16
    dt_out = nc.dram_tensor("dt", [KS, BATCH], fp16, kind="ExternalOutput").ap()

    with tile.TileContext(nc) as tc:
        with (
            tc.tile_pool(name="const", bufs=1) as constp,
            tc.tile_pool(name="xbp", bufs=1) as xbp,
            tc.tile_pool(name="xtp", bufs=1) as xtp,
            tc.tile_pool(name="x2tp", bufs=1) as x2tp,
            tc.tile_pool(name="smal", bufs=1) as smal,
            tc.tile_pool(name="psum", bufs=1, space="PSUM") as psp,
        ):
            # ---- constants (all on DVE; Pool/GPSIMD stays fully idle)
            wtile = constp.tile([128, 512], bf16, tag="wtile")
            nc.vector.memset(wtile[:, :], 0.0)
            ones_col = constp.tile([128, 1], bf16, tag="ones")
            nc.vector.memset(ones_col[:, :], 1.0)

            # ---- DMA in.  One HWDGE stream, engine-bound; order tuned so
            # each consumer ladder unlocks earliest: mask source first (its
            # is_gt gates cmt), then centroid operands, xt last.
            ms_sb = smal.tile([128, 512], bf16, tag="ms")
            nc.sync.dma_start(ms_sb[:, :], ms[:, :])

            xba_sb = xbp.tile([128, 2 * IN_DIM + 256], fp8, tag="xba")
            nc.sync.dma_start(xba_sb[:, :], xba[:, :])
            xb_t = [xba_sb, None]
            us_sb = xba_sb  # U_s lives at cols [2048, 2304)

            xt_q = [xtp.tile([128, 2 * BATCH], bf16, tag=f"xtq{q}", name=f"xtq{q}")
                    for q in range(3)]
            xt_s = [xtp.tile([128, BATCH], bf16, tag=f"xts{j}", name=f"xts{j}")
                    for j in (6, 7)]

            def dma_xtq(q):
                nc.sync.dma_start(
                    xt_q[q][:, :].rearrange("p (r b) -> p r b", r=2),
                    xt[256 * q:256 * (q + 1), :].rearrange("(r p) b -> p r b", p=128),
                )

            dma_xtq(0)
            xbb_sb = xbp.tile([128, 2 * IN_DIM], fp8, tag="xbb")
            nc.sync.dma_start(xbb_sb[:, :], xbb[:, :])
            xb_t[1] = xbb_sb
            dma_xtq(1)
            dma_xtq(2)
            for idx, j in enumerate((6, 7)):
                nc.sync.dma_start(xt_s[idx][:, :], xt[128 * j:128 * (j + 1), :])

            def xt_slice(j):
                if j < 6:
                    return xt_q[j // 2][:, 512 * (j % 2):512 * (j % 2 + 1)]
                return xt_s[j - 6][:, :]

            # ---- PE warm-up: dummy matmuls (no data deps) ramp the PE clock
            # while DMAs land; they write psum_d which T1-j0 later resets.
            psum_d = psp.tile([64, 512], fp32, tag="pd")
            psum_w = psp.tile([64, 512], fp32, tag="pw")

            def dummy_mm(n=512):
                nc.tensor.matmul(psum_w[:, 0:n], wtile[:, 0:64], wtile[:, 0:n],
                                 start=True, stop=True)

            for _ in range(5):
                dummy_mm()

            # ---- maskᵀ = (Mᵀ > 0.5)
            maskt = smal.tile([128, 512], bf16, tag="maskt")
            nc.vector.tensor_scalar(maskt[:, :], ms_sb[:, :], 0.5, None,
                                    Alu.is_ge)

            # ---- Ĉᵀraw[d,k] direct: per d-chunk j accumulate over b-chunks.
            # lhsT = X[b-chunk, d-chunk] (fp8), rhs = U_s[b-chunk] (fp8).
            # j-major (one pending PSUM accumulation group at a time); each
            # xb half covers 4 whole j-groups, so pacing is preserved.
            psum_ct = [psp.tile([128, 256], fp32, tag=f"pct{x}", name=f"pct{x}")
                       for x in range(2)]
            for j in range(ND):
                a, mm = divmod(j, 4)  # xb half a; j-major within each bank
                base = 1024 * (mm // 2) + 128 * (mm % 2)
                for i in range(NB):
                    nc.tensor.matmul(
                        psum_ct[a][:, 64 * mm:64 * (mm + 1)],
                        xb_t[a][:, base + 256 * i:base + 256 * i + 128],
                        us_sb[:, 2048 + KS * i:2048 + KS * (i + 1)],
                        start=(i == 0), stop=(i == NB - 1),
                    )
            dummy_mm(128)

            # ---- X2ᵀ squares as per-j [128, 512] units (each feeds exactly
            # one T1 matmul) alternating DVE/ACT, plus fused CM/G products.
            x2t_q = [x2tp.tile([128, 2 * BATCH], bf16, tag=f"x2q{q}", name=f"x2q{q}")
                     for q in range(3)]
            x2t_s = [x2tp.tile([128, BATCH], bf16, tag=f"x2s{j}", name=f"x2s{j}")
                     for j in (6, 7)]

            def x2t_slice(j):
                if j < 6:
                    return x2t_q[j // 2][:, 512 * (j % 2):512 * (j % 2 + 1)]
                return x2t_s[j - 6][:, :]

            SQ_ON_ACT = {1, 3, 5}
            for j in range(ND):
                dst, srcap = x2t_slice(j), xt_slice(j)
                if j in SQ_ON_ACT:
                    nc.scalar.activation(dst, srcap, Act.Square)
                else:
                    nc.vector.tensor_tensor(dst, srcap, srcap, Alu.mult)

            cmt = smal.tile([128, 512], bf16, tag="cmt")
            g_sb = smal.tile([128, 512], bf16, tag="g")
            for hh in range(2):
                sl = slice(256 * hh, 256 * (hh + 1))
                nc.vector.scalar_tensor_tensor(cmt[:, sl], psum_ct[hh][:, :],
                                               -1.0 / 256.0, maskt[:, sl],
                                               Alu.mult, Alu.mult)
            for hh in range(2):
                sl = slice(256 * hh, 256 * (hh + 1))
                nc.vector.scalar_tensor_tensor(g_sb[:, sl], psum_ct[hh][:, :],
                                               -1.0 / 256.0, cmt[:, sl],
                                               Alu.mult, Alu.mult)

            # ---- Dᵀ accumulation: one PSUM group; T2 (rhs = xt directly)
            # leads since cmt unlocks before the squares; T1-j follows its
            # square.  t3 = colsum(G)/4 accumulates separately as a [64, 1]
            # PSUM vector via near-free N=1 matmuls and is folded into the
            # final PSUM->SBUF copy as a per-partition scalar add.
            def t1(j, start=False, stop=False):
                nc.tensor.matmul(psum_d[:, :], maskt[:, 64 * j:64 * (j + 1)],
                                 x2t_slice(j), start=start, stop=stop)

            def t2(j, start=False, stop=False):
                nc.tensor.matmul(psum_d[:, :], cmt[:, 64 * j:64 * (j + 1)],
                                 xt_slice(j), start=start, stop=stop)

            psum_t3 = psp.tile([64, 1], fp32, tag="pt3")
            d_sb = smal.tile([64, 512], fp16, tag="d")
            t3s = smal.tile([64, 1], fp32, tag="t3s")

            t2(0, start=True)
            t2(1)
            t2(2)
            t2(3)
            t1(0)
            t2(4)
            t2(5)
            t1(1)
            t1(2)
            for j in range(ND):
                nc.tensor.matmul(psum_t3[:, :], g_sb[:, 64 * j:64 * (j + 1)],
                                 ones_col[:, :],
                                 start=(j == 0), stop=(j == ND - 1))
            t1(3)
            t2(6)
            t2(7)
            t1(4)
            t1(5)
            t1(6)
            t1(7, stop=True)

            nc.scalar.activation(t3s[:, :], psum_t3[:, :], Act.Copy, scale=0.25)
            nc.vector.tensor_scalar(d_sb[:, :], psum_d[:, :], t3s[:, 0:1], None,
                                    Alu.add)
            nc.sync.dma_start(dt_out[:, :], d_sb[:, :])

    nc.compile()
    _CACHE[num_devices] = nc
    return nc


def kernel(X: np.ndarray, U: np.ndarray, M: np.ndarray) -> np.ndarray:
    import ml_dtypes
    from concourse import bass_utils

    X = np.asarray(X, dtype=np.float32)
    U = np.asarray(U, dtype=np.float32)
    M = np.asarray(M, dtype=np.float32)
    assert X.shape == (BATCH, IN_DIM) and U.shape == (BATCH, OUT_DIM) \
        and M.shape == (OUT_DIM, IN_DIM)

    nc = build_module(N_CORES)

    bf16 = ml_dtypes.bfloat16
    fp8 = ml_dtypes.float8_e4m3
    # d-chunk-major fp8 layout: [p, 1024*m + 256*i + dd] = X[128*i + p, 256*m + dd]
    xbj = X.reshape(4, 128, 4, 256).transpose(1, 2, 0, 3).reshape(128, 4096)
    xbb_np = np.ascontiguousarray(xbj[:, 2048:4096]).astype(fp8)
    xt_np = np.ascontiguousarray(X.T).astype(bf16)
    def trunc_bf16(a):
        # round-toward-zero to bf16 so (v >= 0.5) == (a > 0.5); exact-0.5
        # inputs (mask must be 0 there) get nudged one bf16 ulp down.
        bits = np.ascontiguousarray(a, dtype=np.float32).view(np.uint32)
        v = (bits >> 16).astype(np.uint16).view(bf16).copy()
        v[a == 0.5] = np.float32(0.498046875)
        return v

    mst = [trunc_bf16(
        M[KS * c:KS * (c + 1), :].T.reshape(8, 128, KS)
        .transpose(1, 0, 2).reshape(128, 512))
        for c in range(N_CORES)]

    in_maps = []
    for c in range(N_CORES):
        usc = U[:, KS * c:KS * (c + 1)].reshape(4, 128, KS).transpose(1, 0, 2)
        xba_np = np.concatenate(
            [xbj[:, 0:2048], usc.reshape(128, 4 * KS)], axis=1).astype(fp8)
        in_maps.append({
            "xba": np.ascontiguousarray(xba_np),
            "xbb": xbb_np,
            "xt": xt_np,
            "ms": mst[c],
        })

    res = bass_utils.run_bass_kernel_spmd(nc, in_maps, core_ids=list(range(N_CORES)))

    out = np.empty((BATCH, OUT_DIM), dtype=np.float32)
    for c in range(N_CORES):
        out[:, KS * c:KS * (c + 1)] = res.results[c]["dt"].T.astype(np.float32)
    return out
